# revision 23
# baseline (speedup 1.0000x reference)
"""GQA attention + RoPE + O-proj, tensor-parallel over 8 NeuronCores.

Strategy (head-parallel TP + all-to-all reshard before O-proj), bf16
matmul inputs with fp32 PSUM accumulation throughout:
  - host: transpose x -> xT [DIM, T] bf16; shuffle per-head wq/wk columns
    to [even hd | odd hd] so RoPE works in the transposed layout; weights
    pre-tiled to SBUF layout so each weight is one contiguous DMA.
  - K/V dedup: each core computes K,V for ONE batch only (host passes
    that batch's tokens as xkvT; even cores get batch 0, odd batch 1),
    ropes K, then a pairwise AllGather (groups {2c, 2c+1}) exchanges the
    halves so both cores of a pair hold full roped K and V. This halves
    per-core K/V projection work; the exchange hides under Q projection.
  - Q projection: weight-stationary bf16 matmuls, xT streamed in
    1024-token pair tiles; RoPE applied inline (sign-folded) on drains.
  - Attention per (local head, batch) in S^T [k, q] layout one 512-wide
    q-group at a time with no-max softmax (scores ~N(0,1)): scores ->
    exp on ACT -> bf16 P strips; diagonal strips are zero-padded
    (gpsimd memset) + tri-masked so softmax denominators accumulate in a
    PSUM bank via one ones-matmul per QUAD of strips (strips pre-summed
    in bf16 on DVE); P@V accumulates O^T per group; fast Newton
    reciprocal normalizes.
  - Two AllToAlls (one per local head) reshard attention outputs
    head-major -> token-sharded; wo quarters + the first AllToAll's recv
    tiles are DMA'd on otherwise-idle engine queues as soon as available
    so the head-0 half of the O-projection (bf16 SBUF partials) starts
    the moment head-1 attention drains and fully covers the second
    AllToAll; the head-1 half then finishes and adds partials.
"""

import os
import numpy as np
import ml_dtypes

import concourse.bass as bass
import concourse.bacc as bacc
import concourse.tile as tile
from concourse import mybir
from concourse.bass_utils import run_bass_kernel_spmd

F32 = mybir.dt.float32
BF16 = mybir.dt.bfloat16

N_CORES = 8

# Full-problem config (hardcoded per spec).
B, SB, DIM = 2, 2048, 2048         # batches, seq per batch, model dim
H, HKV, HD = 16, 4, 128            # q heads, kv heads, head dim
SCALE = 1.0 / float(np.sqrt(HD))

T = B * SB                          # 4096 flat tokens (batch-major)
TPC = T // N_CORES                  # 512 tokens per core (output shard)
HPC = H // N_CORES                  # 2 q heads per core
QW = HPC * HD                       # 256 q cols per core
NKD = DIM // 128                    # 16 contraction tiles for projections
NG = SB // 512                      # 4 q-groups of 512 per batch
KT = SB // 128                      # 16 k-tiles per batch
NTT = T // 128                      # 32 token tiles total
NHD = (H * HD) // 128               # 16 hd row-tiles of wo


def _build():
    nc = bacc.Bacc("TRN2", target_bir_lowering=False, debug=False,
                   num_devices=N_CORES)

    xT = nc.dram_tensor("xT", [DIM, T], BF16, kind="ExternalInput").ap()
    xkvT = nc.dram_tensor("xkvT", [128, NKD, SB], BF16,
                          kind="ExternalInput").ap()
    wq_c = nc.dram_tensor("wq_c", [128, NKD, QW], BF16,
                          kind="ExternalInput").ap()
    wk_c = nc.dram_tensor("wk_c", [128, NKD, HD], BF16,
                          kind="ExternalInput").ap()
    wv_c = nc.dram_tensor("wv_c", [128, NKD, HD], BF16,
                          kind="ExternalInput").ap()
    wo_q = [nc.dram_tensor(f"wo_q{q}", [128, NHD, DIM // 4], BF16,
                           kind="ExternalInput").ap() for q in range(4)]
    cosd = nc.dram_tensor("cosd", [128, SB], BF16, kind="ExternalInput").ap()
    sind = nc.dram_tensor("sind", [128, SB], BF16, kind="ExternalInput").ap()
    sgn = nc.dram_tensor("sgn", [128, 1], F32, kind="ExternalInput").ap()
    tri = nc.dram_tensor("tri", [128, 512], BF16, kind="ExternalInput").ap()
    ones = nc.dram_tensor("ones", [128, 128], BF16, kind="ExternalInput").ap()
    ident = nc.dram_tensor("ident", [128, 128], BF16, kind="ExternalInput").ap()
    out_c = nc.dram_tensor("out_c", [TPC, DIM], BF16,
                           kind="ExternalOutput").ap()

    # pairwise K/V exchange buffers: [k|v][128, SB] in, [rank][k|v] out
    kvx_in = nc.dram_tensor("kvx_in", [2, 128, SB], BF16).ap()
    kvx_out = nc.dram_tensor("kvx_out", [2, 2, 128, SB], BF16).ap()

    a2a_in = []
    a2a_out = []
    for hl in range(HPC):
        a2a_in.append(nc.dram_tensor(f"a2a_in{hl}",
                                     [N_CORES, HD, TPC], BF16).ap())
        a2a_out.append(nc.dram_tensor(f"a2a_out{hl}",
                                      [N_CORES, HD, TPC], BF16).ap())

    SEG = 1024                      # rope segment (never crosses a batch)
    NKQ = 4                         # dim-tiles per xt quarter
    NQT = NKD // NKQ                # quarters per token pair
    DQ = DIM // 4

    with tile.TileContext(nc) as tc:
        with tc.tile_pool(name="const", bufs=1) as constp, \
             tc.tile_pool(name="qkv", bufs=1) as qkvp:
            # persistent roped projections + V in natural layout
            qT0 = qkvp.tile([128, T], BF16, tag="qT0")
            qT1 = qkvp.tile([128, T], BF16, tag="qT1")
            kT = qkvp.tile([128, T], BF16, tag="kT")
            vT = qkvp.tile([128, T], BF16, tag="vT")

            # ---- phase 1: K/V projection (own batch) + exchange ------
            with tc.tile_pool(name="w", bufs=1) as wp, \
                 tc.tile_pool(name="cs", bufs=1) as csp, \
                 tc.tile_pool(name="xkv", bufs=1) as xkvp, \
                 tc.tile_pool(name="kvtmp", bufs=1) as kvtp, \
                 tc.tile_pool(name="xt", bufs=6) as xtp, \
                 tc.tile_pool(name="rtmp", bufs=2) as rp:
                # first-needed-first: wk, wv, then xkv chunks, then wq
                wk_sb = wp.tile([128, NKD * HD], BF16)
                wkv = wk_sb.rearrange("p (n m) -> p n m", n=NKD)
                wv_sb = wp.tile([128, NKD * HD], BF16)
                wvv = wv_sb.rearrange("p (n m) -> p n m", n=NKD)
                nc.sync.dma_start(wkv[:, 0:2], wk_c[:, 0:2, :])
                nc.sync.dma_start(wvv[:, 0:2], wv_c[:, 0:2, :])
                xkv_sb = xkvp.tile([128, NKD * SB], BF16)
                xkv_v = xkv_sb.rearrange("p (n m) -> p n m", n=NKD)
                nc.sync.dma_start(xkv_v[:, 0:1], xkvT[:, 0:1, :])
                nc.scalar.dma_start(xkv_v[:, 1:2], xkvT[:, 1:2, :])
                nc.sync.dma_start(wkv[:, 2:NKD], wk_c[:, 2:NKD, :])
                nc.sync.dma_start(wvv[:, 2:NKD], wv_c[:, 2:NKD, :])
                for qi, (k0, k1) in enumerate(
                        ((2, 4), (4, 6), (6, 8), (8, 10), (10, 12),
                         (12, 14), (14, 16))):
                    eng = nc.scalar if qi % 2 == 0 else nc.sync
                    eng.dma_start(xkv_v[:, k0:k1], xkvT[:, k0:k1, :])
                wq_sb = wp.tile([128, NKD * QW], BF16)
                nc.sync.dma_start(
                    wq_sb.rearrange("p (n m) -> p n m", n=NKD), wq_c[:, :, :])

                # prefetch the first 1.5 pairs of Q-proj x tiles now so the
                # sync queue streams them during the K/V matmuls
                xT3 = xT.rearrange("(n p) m -> p n m", p=128)  # [128,NKD,T]

                def load_xt(q, p):
                    xt_q = xtp.tile([128, NKQ * 1024], BF16, tag="xt",
                                    name=f"xt{q}_{p}")
                    xt_v = xt_q.rearrange("p (n m) -> p n m", n=NKQ)
                    h = NKQ // 2
                    for kq in range(0, NKQ, h):   # halves: separate queues
                        nc.sync.dma_start(
                            xt_v[:, kq:kq + h],
                            xT3[:, q * NKQ + kq:q * NKQ + kq + h,
                                p * 1024:(p + 1) * 1024])
                    return xt_q

                pre = {(q, 0): load_xt(q, 0) for q in range(NQT)}
                for q in range(NQT // 2):
                    pre[(q, 1)] = load_xt(q, 1)

                # small consts + rope tables on side queues
                ident_sb = constp.tile([128, 128], BF16)
                nc.scalar.dma_start(ident_sb[:], ident[:, :])
                sgn_sb = constp.tile([128, 1], F32)
                nc.scalar.dma_start(sgn_sb[:], sgn[:, :])
                cos_sb = csp.tile([128, SB], BF16)
                nc.scalar.dma_start(cos_sb[:], cosd[:, :])
                sin_sb = csp.tile([128, SB], BF16)
                nc.scalar.dma_start(sin_sb[:], sind[:, :])

                def rope(X, s0, pos0, dma_eng):
                    tcs = rp.tile([128, SEG], BF16, tag="tc")
                    nc.vector.tensor_tensor(
                        tcs[:], X[:, s0:s0 + SEG],
                        cos_sb[:, pos0:pos0 + SEG], op=mybir.AluOpType.mult)
                    tsn = rp.tile([128, SEG], BF16, tag="ts")
                    nc.vector.tensor_tensor(
                        tsn[:], X[:, s0:s0 + SEG],
                        sin_sb[:, pos0:pos0 + SEG], op=mybir.AluOpType.mult)
                    tsw = rp.tile([128, SEG], BF16, tag="tw")
                    dma_eng.dma_start(tsw[0:64, :], tsn[64:128, :])
                    dma_eng.dma_start(tsw[64:128, :], tsn[0:64, :])
                    # X = tcs + sgn * tsw   (sgn = -1 top / +1 bottom)
                    nc.vector.scalar_tensor_tensor(
                        X[:, s0:s0 + SEG], tsw[:], sgn_sb[:, 0:1],
                        tcs[:], op0=mybir.AluOpType.mult,
                        op1=mybir.AluOpType.add)

                # K/V matmuls: one pass, all 8 PSUM banks
                with tc.tile_pool(name="pkv", bufs=1, space="PSUM") as pkv:
                    psk = pkv.tile([128, SB], F32, tag="psk")
                    psv = pkv.tile([128, SB], F32, tag="psv")
                    for kk in range(NKD):
                        for ps, wsb in ((psk, wk_sb), (psv, wv_sb)):
                            lhsT = wsb[:, kk * HD:(kk + 1) * HD]
                            for h in range(SB // 512):
                                nc.tensor.matmul(
                                    ps[:, h * 512:(h + 1) * 512], lhsT,
                                    xkv_v[:, kk, h * 512:(h + 1) * 512],
                                    start=(kk == 0), stop=(kk == NKD - 1))
                    vtmp = kvtp.tile([128, SB], BF16, tag="vtmp")
                    nc.scalar.copy(vtmp[:], psv[:])
                    ktmp = kvtp.tile([128, SB], BF16, tag="ktmp")
                    for s0 in range(0, SB, SEG):
                        nc.vector.tensor_copy(ktmp[:, s0:s0 + SEG],
                                              psk[:, s0:s0 + SEG])
                        rope(ktmp, s0, s0, nc.scalar)
                # ship roped K + V to the pair buddy; receive both batches.
                # stores ride the vector/gpsimd queues so the sync queue
                # keeps streaming Q-proj x tiles.
                nc.scalar.dma_start(kvx_in[1, :, :], vtmp[:])
                nc.scalar.dma_start(kvx_in[0, :, :], ktmp[:])
                nc.gpsimd.collective_compute(
                    "AllGather", mybir.AluOpType.bypass,
                    replica_groups=[[2 * i, 2 * i + 1]
                                    for i in range(N_CORES // 2)],
                    ins=[kvx_in.opt()], outs=[kvx_out.opt()])
                for r in range(2):
                    nc.scalar.dma_start(kT[:, r * SB:(r + 1) * SB],
                                        kvx_out[r, 0, :, :])
                    nc.scalar.dma_start(vT[:, r * SB:(r + 1) * SB],
                                        kvx_out[r, 1, :, :])

                # ---- phase 2: Q projection over all tokens -----------
                npair = T // 1024
                with tc.tile_pool(name="pq", bufs=2, space="PSUM") as pq:
                    for p in range(npair):
                        xts = [pre.pop((q, p)) if (q, p) in pre
                               else load_xt(q, p) for q in range(NQT)]
                        pss = [pq.tile([128, 1024], F32, tag=f"pq{c}",
                                       name=f"pq{c}_{p}") for c in range(2)]
                        for kk in range(NKD):
                            xv = xts[kk // NKQ].rearrange(
                                "p (n m) -> p n m", n=NKQ)
                            for c in range(2):
                                lhsT = wq_sb[:, kk * QW + c * 128:
                                             kk * QW + (c + 1) * 128]
                                for j in (0, 1):
                                    nc.tensor.matmul(
                                        pss[c][:, j * 512:(j + 1) * 512],
                                        lhsT,
                                        xv[:, kk % NKQ, j * 512:(j + 1) * 512],
                                        start=(kk == 0), stop=(kk == NKD - 1))
                        cp0 = 1024 * p
                        nc.vector.tensor_copy(qT0[:, cp0:cp0 + 1024], pss[0][:])
                        nc.vector.tensor_copy(qT1[:, cp0:cp0 + 1024],
                                              pss[1][:])
                        pos0 = cp0 % SB
                        rope(qT0, cp0, pos0, nc.sync)
                        rope(qT1, cp0, pos0, nc.sync)

            # ---------------- phase 3: attention ----------------------
            wop = tc.alloc_tile_pool(name="wop", bufs=4)
            op = tc.alloc_tile_pool(name="oproj", bufs=1)
            with tc.tile_pool(name="att", bufs=2) as ap, \
                 tc.tile_pool(name="attc", bufs=1) as apc, \
                 tc.tile_pool(name="pstr", bufs=12) as pstr, \
                 tc.tile_pool(name="psS", bufs=4, space="PSUM") as psS, \
                 tc.tile_pool(name="psD", bufs=2, space="PSUM") as psD, \
                 tc.tile_pool(name="psO", bufs=2, space="PSUM") as psO:
                # wo quarters: single-DMA each, spread over idle queues
                wos = []
                for q in range(4):
                    w = wop.tile([128, NHD * DQ], BF16, tag="wo",
                                 name=f"wo{q}")
                    eng = (nc.sync, nc.scalar, nc.gpsimd, nc.scalar)[q]
                    eng.dma_start(
                        w.rearrange("p (n m) -> p n m", n=NHD), wo_q[q][:, :, :])
                    wos.append(w)
                tri_sb = apc.tile([128, 512], BF16)
                nc.gpsimd.dma_start(tri_sb[:], tri[:, :])
                ones_sb = apc.tile([128, 128], BF16)
                nc.gpsimd.dma_start(ones_sb[:], ones[:, :])
                Vt = qkvp.tile([128, T], BF16, tag="Vt")
                for ttg in range(NTT):
                    psv2 = psS.tile([128, 128], BF16, tag="S")
                    nc.tensor.transpose(psv2[:],
                                        vT[:, ttg * 128:(ttg + 1) * 128],
                                        ident_sb[:])
                    nc.scalar.copy(Vt[:, ttg * 128:(ttg + 1) * 128],
                                   psv2[:])

                recv = {}

                def emit_recv(kks, eng):
                    for kk in kks:
                        rv = op.tile([128, TPC], BF16, tag=f"rv{kk}",
                                     name=f"rv{kk}")
                        eng.dma_start(
                            rv[:], a2a_out[kk % HPC][kk // HPC, :, :])
                        recv[kk] = rv

                for hl in range(HPC):
                    qTh = qT0 if hl == 0 else qT1
                    for b in range(B):
                        if hl == 1 and b == 1:
                            # head-0 recv tiles on the idle gpsimd queue: it
                            # parks on the a2a0-done sem without blocking
                            # anything, so the O-proj A half is ready the
                            # moment attention ends
                            emit_recv(range(0, NHD, HPC), nc.gpsimd)
                        qb = b * SB     # q-col base for this batch
                        # flattened (q-group, k-tile) work list; q-group at
                        # a time so denominators accumulate in a [128,512]
                        # PSUM bank via one ones-matmul per quad of strips.
                        work = [(g, t) for g in range(NG)
                                for t in range(4 * g + 4)]
                        pOs, psrs, Ps, qlos, quad = {}, {}, {}, {}, {}

                        def emit_scores(i, hl=hl, b=b, qb=qb, work=work,
                                        pOs=pOs, psrs=psrs, Ps=Ps, qlos=qlos,
                                        qTh=qTh):
                            g, t = work[i]
                            qlo = 128 * (t - 4 * g) if t >= 4 * g else 0
                            w = 512 - qlo
                            if t == 0:
                                pOs[g] = psO.tile([128, 512], F32, tag="O",
                                                  name=f"pO{hl}{b}{g}")
                                psrs[g] = psD.tile([128, 512], F32, tag="D",
                                                   name=f"psr{hl}{b}{g}")
                            S = psS.tile([128, 512], F32, tag="S")
                            nc.tensor.matmul(
                                S[:, 0:w],
                                kT[:, qb + 128 * t: qb + 128 * (t + 1)],
                                qTh[:, qb + 512 * g + qlo:
                                    qb + 512 * (g + 1)],
                                start=True, stop=True)
                            P = pstr.tile([128, 512], BF16, tag="P")
                            nc.scalar.activation(
                                P[:, qlo:512], S[:, 0:w],
                                mybir.ActivationFunctionType.Exp, scale=SCALE)
                            if t >= 4 * g:     # diagonal tile: causal mask
                                nc.vector.tensor_tensor(
                                    P[:, qlo:512], P[:, qlo:512],
                                    tri_sb[:, 0:w], op=mybir.AluOpType.mult)
                            Ps[i], qlos[i] = P, qlo

                        def emit_accum(i, hl=hl, b=b, work=work, pOs=pOs,
                                       psrs=psrs, Ps=Ps, qlos=qlos,
                                       quad=quad):
                            g, t = work[i]
                            qlo = qlos.pop(i)
                            w = 512 - qlo
                            P = Ps.pop(i)
                            last = (t == 4 * g + 3)
                            nc.tensor.matmul(
                                pOs[g][:, qlo:512],
                                Vt[:, (b * KT + t) * 128:
                                   (b * KT + t + 1) * 128],
                                P[:, qlo:512],
                                start=(t == 0), stop=last,
                                skip_group_check=True)
                            # denominator: accumulate quads of strips in
                            # bf16 on DVE (diagonal strips add only their
                            # valid [qlo:] region); one ones-matmul per quad.
                            qi, pos = t // 4, t % 4
                            if pos == 0:
                                quad[g] = P
                            else:
                                nc.vector.tensor_tensor(
                                    quad[g][:, qlo:512], quad[g][:, qlo:512],
                                    P[:, qlo:512], op=mybir.AluOpType.add)
                            if pos == 3:
                                nc.tensor.matmul(
                                    psrs[g][:], ones_sb[:], quad.pop(g)[:],
                                    start=(qi == 0), stop=(qi == g),
                                    skip_group_check=True)
                            if last:   # group done: normalize + ship
                                rb = ap.tile([128, 512], F32, tag="rb")
                                scr = ap.tile([128, 512], F32, tag="scr")
                                nc.vector.reciprocal_approx_accurate(
                                    rb[:], psrs[g][:], scr[:])
                                Ofin = ap.tile([128, 512], BF16, tag="Of")
                                nc.vector.tensor_tensor(
                                    Ofin[:], pOs[g][:], rb[:],
                                    op=mybir.AluOpType.mult)
                                nc.sync.dma_start(
                                    a2a_in[hl][b * NG + g, :, :], Ofin[:])

                        for i in range(len(work)):
                            emit_scores(i)
                            if i > 1:
                                emit_accum(i - 2)
                        emit_accum(len(work) - 2)
                        emit_accum(len(work) - 1)
                    # per-head collective, overlaps the next head's attention
                    nc.gpsimd.collective_compute(
                        "AllToAll", mybir.AluOpType.bypass,
                        replica_groups=[list(range(N_CORES))],
                        ins=[a2a_in[hl].opt()], outs=[a2a_out[hl].opt()])
                # head-1 recv tiles: ACT is idle during O-proj, so parking
                # its queue on the a2a1-gated loads costs nothing
                emit_recv(range(1, NHD, HPC), nc.scalar)

        # ---------------- phase 5: O-projection ----------------------
            kks0 = list(range(0, NHD, HPC))      # head-0 hd tiles
            kks1 = list(range(1, NHD, HPC))      # head-1 hd tiles
            NQO = DIM // DQ
            NTO = TPC // 128
            with tc.tile_pool(name="opa", bufs=16) as opa, \
                 tc.tile_pool(name="ostg", bufs=4) as ostg, \
                 tc.tile_pool(name="psop", bufs=4, space="PSUM") as pso:
                # phase A: head-0 contributions only (needs just the first
                # AllToAll) -> bf16 partials in SBUF; covers the second.
                pA = {}
                for qp in range(NQO // 2):
                    qs = (2 * qp, 2 * qp + 1)
                    for tt in range(NTO):
                        pos = {}
                        for q in qs:
                            pos[q] = pso.tile([128, DQ], F32, tag="po",
                                              name=f"poA{q}{tt}")
                        for ki, kk in enumerate(kks0):
                            lhs = recv[kk][:, tt * 128:(tt + 1) * 128]
                            for q in qs:
                                nc.tensor.matmul(
                                    pos[q][:], lhs,
                                    wos[q][:, kk * DQ:(kk + 1) * DQ],
                                    start=(ki == 0),
                                    stop=(ki == len(kks0) - 1),
                                    skip_group_check=True)
                        for q in qs:
                            pa = opa.tile([128, DQ], BF16, tag="pa",
                                          name=f"pa{q}{tt}")
                            nc.vector.tensor_copy(pa[:], pos[q][:])
                            pA[(q, tt)] = pa
                # phase B: head-1 contributions + combine + merged store
                for tt in range(NTO):
                    stg = ostg.tile([128, DIM], BF16, tag="stg")
                    for qp in range(NQO // 2):
                        qs = (2 * qp, 2 * qp + 1)
                        pos = {}
                        for q in qs:
                            pos[q] = pso.tile([128, DQ], F32, tag="po",
                                              name=f"poB{q}{tt}")
                        for ki, kk in enumerate(kks1):
                            lhs = recv[kk][:, tt * 128:(tt + 1) * 128]
                            for q in qs:
                                nc.tensor.matmul(
                                    pos[q][:], lhs,
                                    wos[q][:, kk * DQ:(kk + 1) * DQ],
                                    start=(ki == 0),
                                    stop=(ki == len(kks1) - 1),
                                    skip_group_check=True)
                        for q in qs:
                            nc.vector.tensor_tensor(
                                stg[:, q * DQ:(q + 1) * DQ], pos[q][:],
                                pA[(q, tt)][:], op=mybir.AluOpType.add)
                    eng = nc.sync if tt % 2 == 0 else nc.scalar
                    eng.dma_start(out_c[tt * 128:(tt + 1) * 128, :], stg[:])
            op.release()
            wop.release()

    if not nc.is_finalized():
        nc.finalize()
    return nc


_NC_CACHE = {}


def _get_nc():
    if "nc" not in _NC_CACHE:
        _NC_CACHE["nc"] = _build()
    return _NC_CACHE["nc"]


def _prep_inputs(x, cos, sin, wq, wk, wv, wo):
    x = np.asarray(x, np.float32)
    cos = np.asarray(cos, np.float32)
    sin = np.asarray(sin, np.float32)
    wq = np.asarray(wq, np.float32)
    wk = np.asarray(wk, np.float32)
    wv = np.asarray(wv, np.float32)
    wo = np.asarray(wo, np.float32)

    bf16 = ml_dtypes.bfloat16
    xT = np.ascontiguousarray(x.reshape(T, DIM).T).astype(bf16)
    perm = np.r_[np.arange(0, HD, 2), np.arange(1, HD, 2)]
    wq_sh = wq.reshape(DIM, H, HD)[:, :, perm].astype(bf16)
    wk_sh = wk.reshape(DIM, HKV, HD)[:, :, perm].astype(bf16)
    wv_r = wv.reshape(DIM, HKV, HD).astype(bf16)
    cosT = np.ascontiguousarray(cos.T)          # [64, SB]
    cosd_a = np.vstack([cosT, cosT]).astype(bf16)   # [128, SB]
    sinT = np.ascontiguousarray(sin.T)
    sind_a = np.vstack([sinT, sinT]).astype(bf16)
    sgn_a = np.vstack([np.full((64, 1), -1.0, np.float32),
                       np.full((64, 1), 1.0, np.float32)])
    tri_a = (np.arange(512)[None, :] >= np.arange(128)[:, None]
             ).astype(bf16)
    ones_a = np.ones((128, 128), bf16)
    ident_a = np.eye(128, dtype=bf16)
    wo_b = wo.astype(bf16)

    def sbuf_tiled(warr, mcols):
        # [DIM, m] -> [128, NKD, m] so SBUF layout is one contiguous DMA
        return np.ascontiguousarray(
            warr.reshape(NKD, 128, mcols).transpose(1, 0, 2))

    wo_qs = [sbuf_tiled(np.ascontiguousarray(wo_b[:, q * (DIM // 4):
                                                  (q + 1) * (DIM // 4)]),
                        DIM // 4) for q in range(4)]

    in_maps = []
    for c in range(N_CORES):
        h0 = HPC * c
        g = h0 // (H // HKV)
        bsel = c % 2                # even cores project batch 0, odd batch 1
        xkv = np.ascontiguousarray(
            xT[:, bsel * SB:(bsel + 1) * SB].reshape(NKD, 128, SB)
            .transpose(1, 0, 2))
        in_maps.append({
            "xT": xT,
            "xkvT": xkv,
            "wq_c": sbuf_tiled(np.ascontiguousarray(
                wq_sh[:, h0:h0 + HPC].reshape(DIM, QW)), QW),
            "wk_c": sbuf_tiled(np.ascontiguousarray(wk_sh[:, g]), HD),
            "wv_c": sbuf_tiled(np.ascontiguousarray(wv_r[:, g]), HD),
            "wo_q0": wo_qs[0], "wo_q1": wo_qs[1],
            "wo_q2": wo_qs[2], "wo_q3": wo_qs[3],
            "cosd": cosd_a, "sind": sind_a, "sgn": sgn_a, "tri": tri_a,
            "ones": ones_a, "ident": ident_a,
        })
    return in_maps


def _run(inputs, trace=False):
    in_maps = _prep_inputs(**inputs)
    nc = _get_nc()
    res = run_bass_kernel_spmd(
        nc, in_maps, core_ids=list(range(N_CORES)), trace=trace,
        trace_cores=list(range(N_CORES)) if trace else None)
    out = np.concatenate([np.asarray(res.results[c]["out_c"])
                          .astype(np.float32) for c in range(N_CORES)],
                         axis=0)
    return out.reshape(B, SB, DIM), res


def kernel(**inputs):
    out, _ = _run(inputs, trace=os.environ.get("KERNEL_TRACE", "0") == "1")
    return out


# revision 24
# speedup vs baseline: 1.0863x; 1.0863x over previous
"""GQA attention + RoPE + O-proj, tensor-parallel over 8 NeuronCores.

Strategy (head-parallel TP + all-to-all reshard before O-proj), bf16
matmul inputs with fp32 PSUM accumulation throughout:
  - host: transpose x -> xT [DIM, T] bf16; shuffle per-head wq/wk columns
    to [even hd | odd hd] so RoPE works in the transposed layout; weights
    pre-tiled to SBUF layout so each weight is one contiguous DMA.
  - K/V dedup: each core computes K,V for ONE batch only (host passes
    that batch's tokens as xkvT; even cores get batch 0, odd batch 1),
    ropes K, then a pairwise AllGather (groups {2c, 2c+1}) exchanges the
    halves so both cores of a pair hold full roped K and V. This halves
    per-core K/V projection work; the exchange hides under Q projection.
  - Q projection: weight-stationary bf16 matmuls, xT streamed in
    1024-token pair tiles; RoPE applied inline (sign-folded) on drains.
  - Attention per (local head, batch) in S^T [k, q] layout one 512-wide
    q-group at a time with no-max softmax (scores ~N(0,1)): scores ->
    exp on ACT -> bf16 P strips; diagonal strips are zero-padded
    (gpsimd memset) + tri-masked so softmax denominators accumulate in a
    PSUM bank via one ones-matmul per QUAD of strips (strips pre-summed
    in bf16 on DVE); P@V accumulates O^T per group; fast Newton
    reciprocal normalizes.
  - Two AllToAlls (one per local head) reshard attention outputs
    head-major -> token-sharded; wo quarters + the first AllToAll's recv
    tiles are DMA'd on otherwise-idle engine queues as soon as available
    so the head-0 half of the O-projection (bf16 SBUF partials) starts
    the moment head-1 attention drains and fully covers the second
    AllToAll; the head-1 half then finishes and adds partials.
"""

import os
import numpy as np
import ml_dtypes

import concourse.bass as bass
import concourse.bacc as bacc
import concourse.tile as tile
from concourse import mybir
from concourse.bass_utils import run_bass_kernel_spmd

F32 = mybir.dt.float32
BF16 = mybir.dt.bfloat16

N_CORES = 8

# Full-problem config (hardcoded per spec).
B, SB, DIM = 2, 2048, 2048         # batches, seq per batch, model dim
H, HKV, HD = 16, 4, 128            # q heads, kv heads, head dim
SCALE = 1.0 / float(np.sqrt(HD))

T = B * SB                          # 4096 flat tokens (batch-major)
TPC = T // N_CORES                  # 512 tokens per core (output shard)
HPC = H // N_CORES                  # 2 q heads per core
QW = HPC * HD                       # 256 q cols per core
NKD = DIM // 128                    # 16 contraction tiles for projections
NG = SB // 512                      # 4 q-groups of 512 per batch
KT = SB // 128                      # 16 k-tiles per batch
NTT = T // 128                      # 32 token tiles total
NHD = (H * HD) // 128               # 16 hd row-tiles of wo


def _build():
    nc = bacc.Bacc("TRN2", target_bir_lowering=False, debug=False,
                   num_devices=N_CORES)

    xT = nc.dram_tensor("xT", [DIM, T], BF16, kind="ExternalInput").ap()
    xkvT = nc.dram_tensor("xkvT", [128, NKD, SB], BF16,
                          kind="ExternalInput").ap()
    wq_c = nc.dram_tensor("wq_c", [128, NKD, QW], BF16,
                          kind="ExternalInput").ap()
    wk_c = nc.dram_tensor("wk_c", [128, NKD, HD], BF16,
                          kind="ExternalInput").ap()
    wv_c = nc.dram_tensor("wv_c", [128, NKD, HD], BF16,
                          kind="ExternalInput").ap()
    wo_q = [nc.dram_tensor(f"wo_q{q}", [128, NHD, DIM // 4], BF16,
                           kind="ExternalInput").ap() for q in range(4)]
    cosd = nc.dram_tensor("cosd", [128, SB], BF16, kind="ExternalInput").ap()
    sind = nc.dram_tensor("sind", [128, SB], BF16, kind="ExternalInput").ap()
    sgn = nc.dram_tensor("sgn", [128, 1], F32, kind="ExternalInput").ap()
    tri = nc.dram_tensor("tri", [128, 512], BF16, kind="ExternalInput").ap()
    ones = nc.dram_tensor("ones", [128, 128], BF16, kind="ExternalInput").ap()
    ident = nc.dram_tensor("ident", [128, 128], BF16, kind="ExternalInput").ap()
    out_c = nc.dram_tensor("out_c", [TPC, DIM], BF16,
                           kind="ExternalOutput").ap()

    # pairwise K/V exchange buffers: [k|v][128, SB] in, [rank][k|v] out
    kvx_in = nc.dram_tensor("kvx_in", [2, 128, SB], BF16).ap()
    kvx_out = nc.dram_tensor("kvx_out", [2, 2, 128, SB], BF16).ap()

    a2a_in = []
    a2a_out = []
    for hl in range(HPC):
        a2a_in.append(nc.dram_tensor(f"a2a_in{hl}",
                                     [N_CORES, HD, TPC], BF16).ap())
        a2a_out.append(nc.dram_tensor(f"a2a_out{hl}",
                                      [N_CORES, HD, TPC], BF16).ap())

    SEG = 1024                      # rope segment (never crosses a batch)
    NKQ = 4                         # dim-tiles per xt quarter
    NQT = NKD // NKQ                # quarters per token pair
    DQ = DIM // 4

    with tile.TileContext(nc) as tc:
        with tc.tile_pool(name="const", bufs=1) as constp, \
             tc.tile_pool(name="qkv", bufs=1) as qkvp:
            # persistent roped projections + V in natural layout
            qT0 = qkvp.tile([128, T], BF16, tag="qT0")
            qT1 = qkvp.tile([128, T], BF16, tag="qT1")
            kT = qkvp.tile([128, T], BF16, tag="kT")
            vT = qkvp.tile([128, T], BF16, tag="vT")

            # ---- phase 1: K/V projection (own batch) + exchange ------
            with tc.tile_pool(name="w", bufs=1) as wp, \
                 tc.tile_pool(name="cs", bufs=1) as csp, \
                 tc.tile_pool(name="xkv", bufs=1) as xkvp, \
                 tc.tile_pool(name="kvtmp", bufs=1) as kvtp, \
                 tc.tile_pool(name="xt", bufs=6) as xtp, \
                 tc.tile_pool(name="rtmp", bufs=2) as rp:
                # first-needed-first: wk, wv, then xkv chunks, then wq
                wk_sb = wp.tile([128, NKD * HD], BF16)
                wkv = wk_sb.rearrange("p (n m) -> p n m", n=NKD)
                wv_sb = wp.tile([128, NKD * HD], BF16)
                wvv = wv_sb.rearrange("p (n m) -> p n m", n=NKD)
                nc.sync.dma_start(wkv[:, 0:2], wk_c[:, 0:2, :])
                nc.sync.dma_start(wvv[:, 0:2], wv_c[:, 0:2, :])
                xkv_sb = xkvp.tile([128, NKD * SB], BF16)
                xkv_v = xkv_sb.rearrange("p (n m) -> p n m", n=NKD)
                nc.sync.dma_start(xkv_v[:, 0:1], xkvT[:, 0:1, :])
                nc.sync.dma_start(xkv_v[:, 1:2], xkvT[:, 1:2, :])
                nc.sync.dma_start(wkv[:, 2:NKD], wk_c[:, 2:NKD, :])
                nc.sync.dma_start(wvv[:, 2:NKD], wv_c[:, 2:NKD, :])
                for k0, k1 in ((2, 4), (4, 6), (6, 9), (9, 12), (12, 16)):
                    nc.sync.dma_start(xkv_v[:, k0:k1], xkvT[:, k0:k1, :])
                wq_sb = wp.tile([128, NKD * QW], BF16)
                nc.sync.dma_start(
                    wq_sb.rearrange("p (n m) -> p n m", n=NKD), wq_c[:, :, :])

                # prefetch the first 1.5 pairs of Q-proj x tiles now so the
                # sync queue streams them during the K/V matmuls
                xT3 = xT.rearrange("(n p) m -> p n m", p=128)  # [128,NKD,T]

                def load_xt(q, p):
                    xt_q = xtp.tile([128, NKQ * 1024], BF16, tag="xt",
                                    name=f"xt{q}_{p}")
                    xt_v = xt_q.rearrange("p (n m) -> p n m", n=NKQ)
                    h = NKQ // 2
                    for kq in range(0, NKQ, h):   # halves: separate queues
                        nc.sync.dma_start(
                            xt_v[:, kq:kq + h],
                            xT3[:, q * NKQ + kq:q * NKQ + kq + h,
                                p * 1024:(p + 1) * 1024])
                    return xt_q

                pre = {(q, 0): load_xt(q, 0) for q in range(NQT)}
                for q in range(NQT // 2):
                    pre[(q, 1)] = load_xt(q, 1)

                # small consts + rope tables on side queues
                ident_sb = constp.tile([128, 128], BF16)
                nc.scalar.dma_start(ident_sb[:], ident[:, :])
                sgn_sb = constp.tile([128, 1], F32)
                nc.scalar.dma_start(sgn_sb[:], sgn[:, :])
                cos_sb = csp.tile([128, SB], BF16)
                nc.scalar.dma_start(cos_sb[:], cosd[:, :])
                sin_sb = csp.tile([128, SB], BF16)
                nc.scalar.dma_start(sin_sb[:], sind[:, :])

                def rope(X, s0, pos0, dma_eng):
                    tcs = rp.tile([128, SEG], BF16, tag="tc")
                    nc.vector.tensor_tensor(
                        tcs[:], X[:, s0:s0 + SEG],
                        cos_sb[:, pos0:pos0 + SEG], op=mybir.AluOpType.mult)
                    tsn = rp.tile([128, SEG], BF16, tag="ts")
                    nc.vector.tensor_tensor(
                        tsn[:], X[:, s0:s0 + SEG],
                        sin_sb[:, pos0:pos0 + SEG], op=mybir.AluOpType.mult)
                    tsw = rp.tile([128, SEG], BF16, tag="tw")
                    dma_eng.dma_start(tsw[0:64, :], tsn[64:128, :])
                    dma_eng.dma_start(tsw[64:128, :], tsn[0:64, :])
                    # X = tcs + sgn * tsw   (sgn = -1 top / +1 bottom)
                    nc.vector.scalar_tensor_tensor(
                        X[:, s0:s0 + SEG], tsw[:], sgn_sb[:, 0:1],
                        tcs[:], op0=mybir.AluOpType.mult,
                        op1=mybir.AluOpType.add)

                # K/V matmuls: one pass, all 8 PSUM banks
                with tc.tile_pool(name="pkv", bufs=1, space="PSUM") as pkv:
                    psk = pkv.tile([128, SB], F32, tag="psk")
                    psv = pkv.tile([128, SB], F32, tag="psv")
                    for kk in range(NKD):
                        for ps, wsb in ((psk, wk_sb), (psv, wv_sb)):
                            lhsT = wsb[:, kk * HD:(kk + 1) * HD]
                            for h in range(SB // 512):
                                nc.tensor.matmul(
                                    ps[:, h * 512:(h + 1) * 512], lhsT,
                                    xkv_v[:, kk, h * 512:(h + 1) * 512],
                                    start=(kk == 0), stop=(kk == NKD - 1))
                    vtmp = kvtp.tile([128, SB], BF16, tag="vtmp")
                    nc.scalar.copy(vtmp[:], psv[:])
                    ktmp = kvtp.tile([128, SB], BF16, tag="ktmp")
                    for s0 in range(0, SB, SEG):
                        nc.vector.tensor_copy(ktmp[:, s0:s0 + SEG],
                                              psk[:, s0:s0 + SEG])
                        rope(ktmp, s0, s0, nc.scalar)
                # ship roped K + V to the pair buddy; receive both batches.
                # stores ride the vector/gpsimd queues so the sync queue
                # keeps streaming Q-proj x tiles.
                nc.scalar.dma_start(kvx_in[1, :, :], vtmp[:])
                nc.scalar.dma_start(kvx_in[0, :, :], ktmp[:])
                nc.gpsimd.collective_compute(
                    "AllGather", mybir.AluOpType.bypass,
                    replica_groups=[[2 * i, 2 * i + 1]
                                    for i in range(N_CORES // 2)],
                    ins=[kvx_in.opt()], outs=[kvx_out.opt()])
                for r in range(2):
                    nc.scalar.dma_start(kT[:, r * SB:(r + 1) * SB],
                                        kvx_out[r, 0, :, :])
                    nc.scalar.dma_start(vT[:, r * SB:(r + 1) * SB],
                                        kvx_out[r, 1, :, :])

                # ---- phase 2: Q projection over all tokens -----------
                npair = T // 1024
                with tc.tile_pool(name="pq", bufs=2, space="PSUM") as pq:
                    for p in range(npair):
                        xts = [pre.pop((q, p)) if (q, p) in pre
                               else load_xt(q, p) for q in range(NQT)]
                        pss = [pq.tile([128, 1024], F32, tag=f"pq{c}",
                                       name=f"pq{c}_{p}") for c in range(2)]
                        for kk in range(NKD):
                            xv = xts[kk // NKQ].rearrange(
                                "p (n m) -> p n m", n=NKQ)
                            for c in range(2):
                                lhsT = wq_sb[:, kk * QW + c * 128:
                                             kk * QW + (c + 1) * 128]
                                for j in (0, 1):
                                    nc.tensor.matmul(
                                        pss[c][:, j * 512:(j + 1) * 512],
                                        lhsT,
                                        xv[:, kk % NKQ, j * 512:(j + 1) * 512],
                                        start=(kk == 0), stop=(kk == NKD - 1))
                        cp0 = 1024 * p
                        nc.vector.tensor_copy(qT0[:, cp0:cp0 + 1024], pss[0][:])
                        nc.vector.tensor_copy(qT1[:, cp0:cp0 + 1024],
                                              pss[1][:])
                        pos0 = cp0 % SB
                        rope(qT0, cp0, pos0, nc.sync)
                        rope(qT1, cp0, pos0, nc.sync)

            # ---------------- phase 3: attention ----------------------
            wop = tc.alloc_tile_pool(name="wop", bufs=4)
            op = tc.alloc_tile_pool(name="oproj", bufs=1)
            with tc.tile_pool(name="att", bufs=2) as ap, \
                 tc.tile_pool(name="attc", bufs=1) as apc, \
                 tc.tile_pool(name="pstr", bufs=10) as pstr, \
                 tc.tile_pool(name="psS", bufs=4, space="PSUM") as psS, \
                 tc.tile_pool(name="psD", bufs=2, space="PSUM") as psD, \
                 tc.tile_pool(name="psO", bufs=2, space="PSUM") as psO:
                # wo quarters: single-DMA each, spread over idle queues
                wos = []
                for q in range(4):
                    w = wop.tile([128, NHD * DQ], BF16, tag="wo",
                                 name=f"wo{q}")
                    eng = (nc.sync, nc.scalar, nc.gpsimd, nc.scalar)[q]
                    eng.dma_start(
                        w.rearrange("p (n m) -> p n m", n=NHD), wo_q[q][:, :, :])
                    wos.append(w)
                tri_sb = apc.tile([128, 512], BF16)
                nc.gpsimd.dma_start(tri_sb[:], tri[:, :])
                ones_sb = apc.tile([128, 128], BF16)
                nc.gpsimd.dma_start(ones_sb[:], ones[:, :])
                Vt = qkvp.tile([128, T], BF16, tag="Vt")
                for ttg in range(NTT):
                    psv2 = psS.tile([128, 128], BF16, tag="S")
                    nc.tensor.transpose(psv2[:],
                                        vT[:, ttg * 128:(ttg + 1) * 128],
                                        ident_sb[:])
                    nc.scalar.copy(Vt[:, ttg * 128:(ttg + 1) * 128],
                                   psv2[:])

                recv = {}

                def emit_recv(kks, eng):
                    for kk in kks:
                        rv = op.tile([128, TPC], BF16, tag=f"rv{kk}",
                                     name=f"rv{kk}")
                        eng.dma_start(
                            rv[:], a2a_out[kk % HPC][kk // HPC, :, :])
                        recv[kk] = rv

                for hl in range(HPC):
                    qTh = qT0 if hl == 0 else qT1
                    for b in range(B):
                        if hl == 1 and b == 1:
                            # head-0 recv tiles on the idle gpsimd queue: it
                            # parks on the a2a0-done sem without blocking
                            # anything, so the O-proj A half is ready the
                            # moment attention ends
                            emit_recv(range(0, NHD, HPC), nc.gpsimd)
                        qb = b * SB     # q-col base for this batch
                        # flattened (q-group, k-tile) work list; q-group at
                        # a time so denominators accumulate in a [128,512]
                        # PSUM bank via one ones-matmul per quad of strips.
                        work = [(g, t) for g in range(NG)
                                for t in range(4 * g + 4)]
                        pOs, psrs, Ps, qlos, quad = {}, {}, {}, {}, {}

                        def emit_scores(i, hl=hl, b=b, qb=qb, work=work,
                                        pOs=pOs, psrs=psrs, Ps=Ps, qlos=qlos,
                                        qTh=qTh):
                            g, t = work[i]
                            qlo = 128 * (t - 4 * g) if t >= 4 * g else 0
                            w = 512 - qlo
                            if t == 0:
                                pOs[g] = psO.tile([128, 512], F32, tag="O",
                                                  name=f"pO{hl}{b}{g}")
                                psrs[g] = psD.tile([128, 512], F32, tag="D",
                                                   name=f"psr{hl}{b}{g}")
                            S = psS.tile([128, 512], F32, tag="S")
                            nc.tensor.matmul(
                                S[:, 0:w],
                                kT[:, qb + 128 * t: qb + 128 * (t + 1)],
                                qTh[:, qb + 512 * g + qlo:
                                    qb + 512 * (g + 1)],
                                start=True, stop=True)
                            P = pstr.tile([128, 512], BF16, tag="P")
                            nc.scalar.activation(
                                P[:, qlo:512], S[:, 0:w],
                                mybir.ActivationFunctionType.Exp, scale=SCALE)
                            if t >= 4 * g:     # diagonal tile: causal mask
                                nc.vector.tensor_tensor(
                                    P[:, qlo:512], P[:, qlo:512],
                                    tri_sb[:, 0:w], op=mybir.AluOpType.mult)
                            Ps[i], qlos[i] = P, qlo

                        def emit_accum(i, hl=hl, b=b, work=work, pOs=pOs,
                                       psrs=psrs, Ps=Ps, qlos=qlos,
                                       quad=quad):
                            g, t = work[i]
                            qlo = qlos.pop(i)
                            w = 512 - qlo
                            P = Ps.pop(i)
                            last = (t == 4 * g + 3)
                            nc.tensor.matmul(
                                pOs[g][:, qlo:512],
                                Vt[:, (b * KT + t) * 128:
                                   (b * KT + t + 1) * 128],
                                P[:, qlo:512],
                                start=(t == 0), stop=last,
                                skip_group_check=True)
                            # denominator: accumulate quads of strips in
                            # bf16 on DVE (diagonal strips add only their
                            # valid [qlo:] region); one ones-matmul per quad.
                            qi, pos = t // 4, t % 4
                            if pos == 0:
                                quad[g] = P
                            else:
                                nc.vector.tensor_tensor(
                                    quad[g][:, qlo:512], quad[g][:, qlo:512],
                                    P[:, qlo:512], op=mybir.AluOpType.add)
                            if pos == 3:
                                nc.tensor.matmul(
                                    psrs[g][:], ones_sb[:], quad.pop(g)[:],
                                    start=(qi == 0), stop=(qi == g),
                                    skip_group_check=True)
                            if last:   # group done: normalize + ship
                                rb = ap.tile([128, 512], F32, tag="rb")
                                scr = ap.tile([128, 512], F32, tag="scr")
                                nc.vector.reciprocal_approx_accurate(
                                    rb[:], psrs[g][:], scr[:])
                                Ofin = ap.tile([128, 512], BF16, tag="Of")
                                nc.vector.tensor_tensor(
                                    Ofin[:], pOs[g][:], rb[:],
                                    op=mybir.AluOpType.mult)
                                nc.sync.dma_start(
                                    a2a_in[hl][b * NG + g, :, :], Ofin[:])

                        for i in range(len(work)):
                            emit_scores(i)
                            if i > 0:
                                emit_accum(i - 1)
                        emit_accum(len(work) - 1)
                    # per-head collective, overlaps the next head's attention
                    nc.gpsimd.collective_compute(
                        "AllToAll", mybir.AluOpType.bypass,
                        replica_groups=[list(range(N_CORES))],
                        ins=[a2a_in[hl].opt()], outs=[a2a_out[hl].opt()])
                # head-1 recv tiles: ACT is idle during O-proj, so parking
                # its queue on the a2a1-gated loads costs nothing
                emit_recv(range(1, NHD, HPC), nc.scalar)

        # ---------------- phase 5: O-projection ----------------------
            kks0 = list(range(0, NHD, HPC))      # head-0 hd tiles
            kks1 = list(range(1, NHD, HPC))      # head-1 hd tiles
            NQO = DIM // DQ
            NTO = TPC // 128
            with tc.tile_pool(name="opa", bufs=16) as opa, \
                 tc.tile_pool(name="ostg", bufs=4) as ostg, \
                 tc.tile_pool(name="psop", bufs=4, space="PSUM") as pso:
                # phase A: head-0 contributions only (needs just the first
                # AllToAll) -> bf16 partials in SBUF; covers the second.
                pA = {}
                for qp in range(NQO // 2):
                    qs = (2 * qp, 2 * qp + 1)
                    for tt in range(NTO):
                        pos = {}
                        for q in qs:
                            pos[q] = pso.tile([128, DQ], F32, tag="po",
                                              name=f"poA{q}{tt}")
                        for ki, kk in enumerate(kks0):
                            lhs = recv[kk][:, tt * 128:(tt + 1) * 128]
                            for q in qs:
                                nc.tensor.matmul(
                                    pos[q][:], lhs,
                                    wos[q][:, kk * DQ:(kk + 1) * DQ],
                                    start=(ki == 0),
                                    stop=(ki == len(kks0) - 1),
                                    skip_group_check=True)
                        for q in qs:
                            pa = opa.tile([128, DQ], BF16, tag="pa",
                                          name=f"pa{q}{tt}")
                            nc.vector.tensor_copy(pa[:], pos[q][:])
                            pA[(q, tt)] = pa
                # phase B: head-1 contributions + combine + store
                for qp in range(NQO // 2):
                    qs = (2 * qp, 2 * qp + 1)
                    for tt in range(NTO):
                        pos = {}
                        for q in qs:
                            pos[q] = pso.tile([128, DQ], F32, tag="po",
                                              name=f"poB{q}{tt}")
                        for ki, kk in enumerate(kks1):
                            lhs = recv[kk][:, tt * 128:(tt + 1) * 128]
                            for q in qs:
                                nc.tensor.matmul(
                                    pos[q][:], lhs,
                                    wos[q][:, kk * DQ:(kk + 1) * DQ],
                                    start=(ki == 0),
                                    stop=(ki == len(kks1) - 1),
                                    skip_group_check=True)
                        stg = ostg.tile([128, 2 * DQ], BF16, tag="stg")
                        for qi2, q in enumerate(qs):
                            nc.vector.tensor_tensor(
                                stg[:, qi2 * DQ:(qi2 + 1) * DQ], pos[q][:],
                                pA[(q, tt)][:], op=mybir.AluOpType.add)
                        eng = nc.sync if tt % 2 == 0 else nc.scalar
                        eng.dma_start(
                            out_c[tt * 128:(tt + 1) * 128,
                                  qs[0] * DQ:(qs[0] + 2) * DQ], stg[:])
            op.release()
            wop.release()

    if not nc.is_finalized():
        nc.finalize()
    return nc


_NC_CACHE = {}


def _get_nc():
    if "nc" not in _NC_CACHE:
        _NC_CACHE["nc"] = _build()
    return _NC_CACHE["nc"]


def _prep_inputs(x, cos, sin, wq, wk, wv, wo):
    x = np.asarray(x, np.float32)
    cos = np.asarray(cos, np.float32)
    sin = np.asarray(sin, np.float32)
    wq = np.asarray(wq, np.float32)
    wk = np.asarray(wk, np.float32)
    wv = np.asarray(wv, np.float32)
    wo = np.asarray(wo, np.float32)

    bf16 = ml_dtypes.bfloat16
    xT = np.ascontiguousarray(x.reshape(T, DIM).T).astype(bf16)
    perm = np.r_[np.arange(0, HD, 2), np.arange(1, HD, 2)]
    wq_sh = wq.reshape(DIM, H, HD)[:, :, perm].astype(bf16)
    wk_sh = wk.reshape(DIM, HKV, HD)[:, :, perm].astype(bf16)
    wv_r = wv.reshape(DIM, HKV, HD).astype(bf16)
    cosT = np.ascontiguousarray(cos.T)          # [64, SB]
    cosd_a = np.vstack([cosT, cosT]).astype(bf16)   # [128, SB]
    sinT = np.ascontiguousarray(sin.T)
    sind_a = np.vstack([sinT, sinT]).astype(bf16)
    sgn_a = np.vstack([np.full((64, 1), -1.0, np.float32),
                       np.full((64, 1), 1.0, np.float32)])
    tri_a = (np.arange(512)[None, :] >= np.arange(128)[:, None]
             ).astype(bf16)
    ones_a = np.ones((128, 128), bf16)
    ident_a = np.eye(128, dtype=bf16)
    wo_b = wo.astype(bf16)

    def sbuf_tiled(warr, mcols):
        # [DIM, m] -> [128, NKD, m] so SBUF layout is one contiguous DMA
        return np.ascontiguousarray(
            warr.reshape(NKD, 128, mcols).transpose(1, 0, 2))

    wo_qs = [sbuf_tiled(np.ascontiguousarray(wo_b[:, q * (DIM // 4):
                                                  (q + 1) * (DIM // 4)]),
                        DIM // 4) for q in range(4)]

    in_maps = []
    for c in range(N_CORES):
        h0 = HPC * c
        g = h0 // (H // HKV)
        bsel = c % 2                # even cores project batch 0, odd batch 1
        xkv = np.ascontiguousarray(
            xT[:, bsel * SB:(bsel + 1) * SB].reshape(NKD, 128, SB)
            .transpose(1, 0, 2))
        in_maps.append({
            "xT": xT,
            "xkvT": xkv,
            "wq_c": sbuf_tiled(np.ascontiguousarray(
                wq_sh[:, h0:h0 + HPC].reshape(DIM, QW)), QW),
            "wk_c": sbuf_tiled(np.ascontiguousarray(wk_sh[:, g]), HD),
            "wv_c": sbuf_tiled(np.ascontiguousarray(wv_r[:, g]), HD),
            "wo_q0": wo_qs[0], "wo_q1": wo_qs[1],
            "wo_q2": wo_qs[2], "wo_q3": wo_qs[3],
            "cosd": cosd_a, "sind": sind_a, "sgn": sgn_a, "tri": tri_a,
            "ones": ones_a, "ident": ident_a,
        })
    return in_maps


def _run(inputs, trace=False):
    in_maps = _prep_inputs(**inputs)
    nc = _get_nc()
    res = run_bass_kernel_spmd(
        nc, in_maps, core_ids=list(range(N_CORES)), trace=trace,
        trace_cores=list(range(N_CORES)) if trace else None)
    out = np.concatenate([np.asarray(res.results[c]["out_c"])
                          .astype(np.float32) for c in range(N_CORES)],
                         axis=0)
    return out.reshape(B, SB, DIM), res


def kernel(**inputs):
    out, _ = _run(inputs, trace=os.environ.get("KERNEL_TRACE", "0") == "1")
    return out
